# revision 3
# baseline (speedup 1.0000x reference)
"""Batch-parallel attention kernel for 8 TRN2 NeuronCores.

Problem: B=16, S=2048, D=128 full (non-causal) attention, fp32 I/O.
Sharding: batch dim across 8 cores (2 batches/core), no collectives.

Per-core layout trick: everything is computed in "transposed score" space
S^T[k, q] so that no on-device transposes are needed:
  - matmul1: S^T[k,q] = (K^T)[d,k]^T-stationary @ (Q^T)[d,q]-moving,
    contraction over d=128 partitions. Q^T/K^T are prepared on host.
  - ScalarE: expS^T = exp(scale * S^T) PSUM->SBUF (bf16), no max
    subtraction (scores are ~N(0,1); max over dataset ~7.5 -> exp fine).
  - matmul2: out[q, 0:129] = sum_k expS^T[k,q]^T-stationary @ V_aug[k,:]
    where V_aug = [V | ones]; column 128 accumulates the softmax
    denominator exactly in fp32 PSUM.
  - VectorE: reciprocal of the denominator column + per-partition
    tensor_scalar multiply -> normalized out tile, DMA'd out natively.
"""

import os

import ml_dtypes
import numpy as np

import concourse.bass as bass
import concourse.mybir as mybir
import concourse.tile as tile
from concourse import bacc
from concourse.bass_utils import run_bass_kernel_spmd

B, S, D = 16, 2048, 128
N_CORES = 8
BPC = B // N_CORES          # batches per core
DA = D + 1                  # V augmented with ones column
QCHUNK = 512                # q processed per inner pipeline chunk
N_QC = S // QCHUNK          # 4
N_KT = S // 128             # 16 k-tiles
SCALE = 1.0 / float(np.sqrt(D))

BF16 = mybir.dt.bfloat16
F32 = mybir.dt.float32

TRACE = bool(os.environ.get("BASS_KERNEL_TRACE"))
LAST_RESULTS = None

_CACHE = {}


def _build():
    nc = bacc.Bacc("TRN2", target_bir_lowering=False, debug=False)

    qT = nc.dram_tensor("qT", [BPC, D, S], BF16, kind="ExternalInput").ap()
    kT = nc.dram_tensor("kT", [BPC, D, S], BF16, kind="ExternalInput").ap()
    vA = nc.dram_tensor("vA", [BPC, S, DA], BF16, kind="ExternalInput").ap()
    out = nc.dram_tensor("out", [BPC, S, D], F32, kind="ExternalOutput").ap()

    with tile.TileContext(nc) as tc:
        with (
            tc.tile_pool(name="qk", bufs=2) as qk_pool,
            tc.tile_pool(name="vp", bufs=2) as v_pool,
            tc.tile_pool(name="pexp", bufs=4) as p_pool,
            tc.tile_pool(name="outs", bufs=8) as o_pool,
            tc.tile_pool(name="psum_s", bufs=2, space="PSUM") as psum_s,
            tc.tile_pool(name="psum_acc", bufs=1, space="PSUM") as psum_acc,
        ):
            for b in range(BPC):
                qT_sb = qk_pool.tile([128, S], BF16, tag="qT")
                kT_sb = qk_pool.tile([128, S], BF16, tag="kT")
                v_sb = v_pool.tile([128, N_KT, DA], BF16)
                nc.sync.dma_start(out=qT_sb, in_=qT[b])
                nc.sync.dma_start(out=kT_sb, in_=kT[b])
                nc.sync.dma_start(
                    out=v_sb, in_=vA[b].rearrange("(t p) d -> p t d", p=128)
                )

                for qc in range(N_QC):
                    qs = qc * QCHUNK
                    acc = [
                        psum_acc.tile(
                            [128, DA], F32, tag=f"acc{j}", name=f"acc{j}"
                        )
                        for j in range(4)
                    ]

                    # software-pipelined: emit m2(kt-1) after m1(kt) so PE
                    # never stalls waiting for the exp of the tile it just
                    # produced.
                    pending = None  # (kt, p_tile) awaiting matmul2

                    def emit_m2(kt, p_tile):
                        for j in range(4):
                            nc.tensor.matmul(
                                acc[j],
                                lhsT=p_tile[:, j * 128 : (j + 1) * 128],
                                rhs=v_sb[:, kt, :],
                                start=(kt == 0),
                                stop=(kt == N_KT - 1),
                            )

                    for kt in range(N_KT):
                        s_psum = psum_s.tile([128, QCHUNK], F32)
                        nc.tensor.matmul(
                            s_psum,
                            lhsT=kT_sb[:, kt * 128 : (kt + 1) * 128],
                            rhs=qT_sb[:, qs : qs + QCHUNK],
                            start=True,
                            stop=True,
                        )
                        p_tile = p_pool.tile([128, QCHUNK], BF16)
                        nc.scalar.activation(
                            p_tile,
                            s_psum,
                            mybir.ActivationFunctionType.Exp,
                            scale=SCALE,
                        )
                        if pending is not None:
                            emit_m2(*pending)
                        pending = (kt, p_tile)
                    emit_m2(*pending)

                    for j in range(4):
                        recip = o_pool.tile([128, 1], F32, tag="recip")
                        nc.vector.reciprocal(recip, acc[j][:, D : D + 1])
                        o_sb = o_pool.tile([128, D], F32, tag="o")
                        nc.vector.tensor_scalar_mul(o_sb, acc[j][:, 0:D], recip)
                        r0 = qs + j * 128
                        nc.sync.dma_start(out=out[b, r0 : r0 + 128, :], in_=o_sb)

    nc.compile()
    return nc


def _get_nc():
    if "nc" not in _CACHE:
        _CACHE["nc"] = _build()
    return _CACHE["nc"]


def kernel(query, key, value):
    global LAST_RESULTS
    bf16 = ml_dtypes.bfloat16
    q = np.ascontiguousarray(
        np.asarray(query, dtype=np.float32).transpose(0, 2, 1)
    ).astype(bf16)
    k = np.ascontiguousarray(
        np.asarray(key, dtype=np.float32).transpose(0, 2, 1)
    ).astype(bf16)
    v = np.asarray(value, dtype=np.float32)
    v_aug = np.concatenate(
        [v, np.ones((B, S, 1), dtype=np.float32)], axis=2
    ).astype(bf16)

    nc = _get_nc()
    in_maps = [
        {
            "qT": q[i * BPC : (i + 1) * BPC],
            "kT": k[i * BPC : (i + 1) * BPC],
            "vA": v_aug[i * BPC : (i + 1) * BPC],
        }
        for i in range(N_CORES)
    ]
    res = run_bass_kernel_spmd(
        nc, in_maps, core_ids=list(range(N_CORES)), trace=TRACE
    )
    LAST_RESULTS = res
    out = np.empty((B, S, D), dtype=np.float32)
    for i in range(N_CORES):
        out[i * BPC : (i + 1) * BPC] = res.results[i]["out"]
    return out


# revision 4
# speedup vs baseline: 1.4228x; 1.4228x over previous
"""Batch-parallel attention kernel for 8 TRN2 NeuronCores.

Problem: B=16, S=2048, D=128 full (non-causal) attention, fp32 I/O.
Sharding: batch dim across 8 cores (2 batches/core), no collectives.

Per-core layout trick: everything is computed in "transposed score" space
S^T[k, q] so that no on-device transposes are needed:
  - matmul1: S^T[k,q] = (K^T)[d,k]^T-stationary @ (Q^T)[d,q]-moving,
    contraction over d=128 partitions. Q^T/K^T are prepared on host.
  - ScalarE: expS^T = exp(scale * S^T) PSUM->SBUF (bf16), no max
    subtraction (scores are ~N(0,1); max over dataset ~7.5 -> exp fine).
  - matmul2: out[q, 0:129] = sum_k expS^T[k,q]^T-stationary @ V_aug[k,:]
    where V_aug = [V | ones]; column 128 accumulates the softmax
    denominator exactly in fp32 PSUM.
  - VectorE: reciprocal of the denominator column + per-partition
    tensor_scalar multiply -> normalized out tile, DMA'd out natively.
"""

import os

import ml_dtypes
import numpy as np

import concourse.bass as bass
import concourse.mybir as mybir
import concourse.tile as tile
from concourse import bacc
from concourse.bass_utils import run_bass_kernel_spmd

B, S, D = 16, 2048, 128
N_CORES = 8
BPC = B // N_CORES          # batches per core
DA = D + 1                  # V augmented with ones column
QCHUNK = 512                # q processed per inner pipeline chunk
N_QC = S // QCHUNK          # 4
N_KT = S // 128             # 16 k-tiles
SCALE = 1.0 / float(np.sqrt(D))

BF16 = mybir.dt.bfloat16
F32 = mybir.dt.float32

TRACE = bool(os.environ.get("BASS_KERNEL_TRACE"))
LAST_RESULTS = None

_CACHE = {}


def _build():
    nc = bacc.Bacc("TRN2", target_bir_lowering=False, debug=False)

    qT = nc.dram_tensor("qT", [BPC, D, S], BF16, kind="ExternalInput").ap()
    kT = nc.dram_tensor("kT", [BPC, D, S], BF16, kind="ExternalInput").ap()
    vA = nc.dram_tensor("vA", [BPC, S, DA], BF16, kind="ExternalInput").ap()
    out = nc.dram_tensor("out", [BPC, S, D], F32, kind="ExternalOutput").ap()

    with tile.TileContext(nc) as tc:
        with (
            tc.tile_pool(name="qk", bufs=2) as qk_pool,
            tc.tile_pool(name="vp", bufs=2) as v_pool,
            tc.tile_pool(name="pexp", bufs=4) as p_pool,
            tc.tile_pool(name="outs", bufs=8) as o_pool,
            tc.tile_pool(name="psum_s", bufs=2, space="PSUM") as psum_s,
            tc.tile_pool(name="psum_acc", bufs=1, space="PSUM") as psum_acc,
        ):
            for b in range(BPC):
                qT_sb = qk_pool.tile([128, S], BF16, tag="qT")
                kT_sb = qk_pool.tile([128, S], BF16, tag="kT")
                v_sb = v_pool.tile([128, N_KT, DA], BF16)
                nc.sync.dma_start(out=qT_sb, in_=qT[b])
                nc.sync.dma_start(out=kT_sb, in_=kT[b])
                nc.sync.dma_start(
                    out=v_sb, in_=vA[b].rearrange("(t p) d -> p t d", p=128)
                )

                for qc in range(N_QC):
                    qs = qc * QCHUNK
                    acc = [
                        psum_acc.tile(
                            [128, DA], F32, tag=f"acc{j}", name=f"acc{j}"
                        )
                        for j in range(4)
                    ]

                    # software-pipelined: emit m2 for the previous kt-pair
                    # after m1 of the current pair so PE never stalls waiting
                    # for the exp of the tile it just produced.
                    pending = None  # (kt0, p_tile) awaiting matmul2

                    def emit_m2(kt0, p_tile):
                        for h in range(2):
                            kt = kt0 + h
                            for j in range(4):
                                nc.tensor.matmul(
                                    acc[j],
                                    lhsT=p_tile[:, h, j * 128 : (j + 1) * 128],
                                    rhs=v_sb[:, kt, :],
                                    start=(kt == 0),
                                    stop=(kt == N_KT - 1),
                                )

                    for kt0 in range(0, N_KT, 2):
                        s_psum = psum_s.tile([128, 2, QCHUNK], F32)
                        for h in range(2):
                            nc.tensor.matmul(
                                s_psum[:, h, :],
                                lhsT=kT_sb[:, (kt0 + h) * 128 : (kt0 + h + 1) * 128],
                                rhs=qT_sb[:, qs : qs + QCHUNK],
                                start=True,
                                stop=True,
                            )
                        p_tile = p_pool.tile([128, 2, QCHUNK], BF16)
                        nc.scalar.activation(
                            p_tile,
                            s_psum,
                            mybir.ActivationFunctionType.Exp,
                            scale=SCALE,
                        )
                        if pending is not None:
                            emit_m2(*pending)
                        pending = (kt0, p_tile)
                    emit_m2(*pending)

                    for j in range(4):
                        recip = o_pool.tile([128, 1], F32, tag="recip")
                        nc.vector.reciprocal(recip, acc[j][:, D : D + 1])
                        o_sb = o_pool.tile([128, D], F32, tag="o")
                        nc.vector.tensor_scalar_mul(o_sb, acc[j][:, 0:D], recip)
                        r0 = qs + j * 128
                        nc.sync.dma_start(out=out[b, r0 : r0 + 128, :], in_=o_sb)

    nc.compile()
    return nc


def _get_nc():
    if "nc" not in _CACHE:
        _CACHE["nc"] = _build()
    return _CACHE["nc"]


def kernel(query, key, value):
    global LAST_RESULTS
    bf16 = ml_dtypes.bfloat16
    q = np.ascontiguousarray(
        np.asarray(query, dtype=np.float32).transpose(0, 2, 1)
    ).astype(bf16)
    k = np.ascontiguousarray(
        np.asarray(key, dtype=np.float32).transpose(0, 2, 1)
    ).astype(bf16)
    v = np.asarray(value, dtype=np.float32)
    v_aug = np.concatenate(
        [v, np.ones((B, S, 1), dtype=np.float32)], axis=2
    ).astype(bf16)

    nc = _get_nc()
    in_maps = [
        {
            "qT": q[i * BPC : (i + 1) * BPC],
            "kT": k[i * BPC : (i + 1) * BPC],
            "vA": v_aug[i * BPC : (i + 1) * BPC],
        }
        for i in range(N_CORES)
    ]
    res = run_bass_kernel_spmd(
        nc, in_maps, core_ids=list(range(N_CORES)), trace=TRACE
    )
    LAST_RESULTS = res
    out = np.empty((B, S, D), dtype=np.float32)
    for i in range(N_CORES):
        out[i * BPC : (i + 1) * BPC] = res.results[i]["out"]
    return out
